# revision 3
# baseline (speedup 1.0000x reference)
"""Trainium2 Bass kernel for hierarchical-classification AWX head.

Computes, for inputs x[B, L] (f32) and 0/1 adjacency R[C, L] (int32):

    o   = sigmoid(x)
    s   = einsum('bl,cl->bc', o**5, R)          (R**5 == R since R is 0/1)
    out = clip(s, EPS, 1-EPS) ** (1/5)

Sharding: R is split row-wise (class dim) across the 8 NeuronCores; each
core computes a [B, C/8] slice of the output against the full (replicated)
x. No cross-device reduction is needed; the host concatenates the slices.

Per-core pipeline (all compute on device):
  - x is DMA'd folded as [128, 2048] ((l-half, b) on partitions).
  - sigmoid(x)^5 = exp(-5 * ln(1 + exp(-x))): 3 ScalarE passes, one ACT
    table set (natural_log_exp_and_others) so no table-switch cost.
  - R slice is cast int32->bf16 during the SWDGE DMA load (values 0/1).
  - The matmul contracts over l, so both operands need l on partitions:
    both are transposed on the TensorE via matmul-with-identity
    (out = tile^T @ I), grouped 4 tiles per PSUM bank, evacuated by
    VectorE/ScalarE copies (f32->bf16).
  - 32 accumulating bf16 matmuls build s[64, 256] in PSUM.
  - Tail: clip (VectorE two-op tensor_scalar), ln, exp(0.2*) (ScalarE).
"""

import numpy as np

B, L, C = 64, 4096, 2048
NCORES = 8
CP = C // NCORES  # 256 classes per core
EPS = 1e-6

NK = L // 128  # 32 contraction chunks of 128
H = 2          # fold factor for x: [64, 4096] -> [128, 2048]
QW = 1024      # R dma chunk width along l
NQ = L // QW   # 4 chunks per 128-row block of R

_STATE = {}


def _build_nc():
    from contextlib import ExitStack

    import ml_dtypes
    import concourse.bacc as bacc
    import concourse.mybir as mybir
    from concourse.tile import TileContext

    dt = mybir.dt
    AF = mybir.ActivationFunctionType
    ALU = mybir.AluOpType

    nc = bacc.Bacc("TRN2", target_bir_lowering=False)

    x_d = nc.dram_tensor("x", [B, L], dt.float32, kind="ExternalInput")
    r_d = nc.dram_tensor("r", [CP, L], dt.int32, kind="ExternalInput")
    o_d = nc.dram_tensor("out", [B, CP], dt.float32, kind="ExternalOutput")
    ident_d = nc.inline_tensor(np.eye(128, dtype=ml_dtypes.bfloat16), "ident")

    with TileContext(nc) as tc, ExitStack() as ctx:
        const = ctx.enter_context(tc.tile_pool(name="const", bufs=1))
        xin = ctx.enter_context(tc.tile_pool(name="xin", bufs=1))
        actp = ctx.enter_context(tc.tile_pool(name="actp", bufs=2))
        o5p = ctx.enter_context(tc.tile_pool(name="o5p", bufs=1))
        otp = ctx.enter_context(tc.tile_pool(name="otp", bufs=4))
        rbp = ctx.enter_context(tc.tile_pool(name="rbp", bufs=4))
        rtp = ctx.enter_context(tc.tile_pool(name="rtp", bufs=4))
        tailp = ctx.enter_context(tc.tile_pool(name="tailp", bufs=3))
        pst = ctx.enter_context(tc.tile_pool(name="pst", bufs=3, space="PSUM"))
        pss = ctx.enter_context(tc.tile_pool(name="pss", bufs=1, space="PSUM"))

        ident = const.tile([128, 128], dt.bfloat16)
        nc.sync.dma_start(out=ident[:], in_=ident_d[:])

        # x folded: partition p = 64*h + b, free q = l % 2048 (l = 2048h + q)
        xf = xin.tile([128, L // H], dt.float32)
        for h in range(H):
            nc.sync.dma_start(
                out=xf[64 * h : 64 * (h + 1), :],
                in_=x_d[:, (L // H) * h : (L // H) * (h + 1)],
            )

        # R slice loads, cast int32 -> bf16 during DMA (SWDGE).
        # rb[(t, q)][c', l'] = R[128t + c', QW*q + l'] for this core's slice.
        rb = {}
        for q in range(NQ):
            for t in range(2):
                tile_ = rbp.tile([128, QW], dt.bfloat16, tag="rb")
                nc.gpsimd.dma_start(
                    out=tile_[:],
                    in_=r_d[128 * t : 128 * (t + 1), QW * q : QW * (q + 1)],
                )
                rb[(t, q)] = tile_

        # o5 = sigmoid(x)^5 = exp(-5 * ln(1 + exp(-x))); Exp and Ln share
        # one ACT table set so only one table load is paid.
        t1 = actp.tile([128, L // H], dt.float32, tag="acttmp")
        nc.scalar.activation(out=t1[:], in_=xf[:], func=AF.Exp, scale=-1.0)
        u = actp.tile([128, L // H], dt.float32, tag="acttmp")
        nc.scalar.activation(out=u[:], in_=t1[:], func=AF.Ln, bias=1.0)
        o5b = o5p.tile([128, L // H], dt.bfloat16)
        nc.scalar.activation(out=o5b[:], in_=u[:], func=AF.Exp, scale=-5.0)

        # Transpose o5 via matmul-with-identity, 4 tiles per PSUM bank.
        # ot[jg] column 128*jj + 64*h + b at partition q' = o5[b, 2048h + 128j + q']
        ot = []
        for jg in range(4):
            ps = pst.tile([128, 512], dt.float32, tag="pst")
            for jj in range(4):
                j = 4 * jg + jj
                nc.tensor.matmul(
                    out=ps[:, 128 * jj : 128 * (jj + 1)],
                    lhsT=o5b[:, 128 * j : 128 * (j + 1)],
                    rhs=ident[:],
                    start=True,
                    stop=True,
                )
            sb = otp.tile([128, 512], dt.bfloat16, tag="ot")
            nc.vector.tensor_copy(out=sb[:], in_=ps[:])
            ot.append(sb)

        # s accumulator: [64 b, 256 c] f32 in one PSUM bank
        s_ps = pss.tile([B, CP], dt.float32)

        # For each pair of contraction chunks: 4 R-transposes into one PSUM
        # bank, one grouped copy to SBUF (bf16), then 2 accumulating matmuls.
        for kk in range(NK // 2):
            ps = pst.tile([128, 512], dt.float32, tag="pst")
            for sub in range(2):
                k = 2 * kk + sub
                q, off = divmod(k * 128, QW)
                for t in range(2):
                    nc.tensor.matmul(
                        out=ps[:, 256 * sub + 128 * t : 256 * sub + 128 * (t + 1)],
                        lhsT=rb[(t, q)][:, off : off + 128],
                        rhs=ident[:],
                        start=True,
                        stop=True,
                    )
            rt = rtp.tile([128, 512], dt.bfloat16, tag="rt")
            if kk % 2 == 0:
                nc.vector.tensor_copy(out=rt[:], in_=ps[:])
            else:
                nc.scalar.copy(out=rt[:], in_=ps[:])
            for sub in range(2):
                k = 2 * kk + sub
                j, h = k % 16, k // 16
                jg, jj = divmod(j, 4)
                nc.tensor.matmul(
                    out=s_ps[:],
                    lhsT=ot[jg][:, 128 * jj + 64 * h : 128 * jj + 64 * h + 64],
                    rhs=rt[:, 256 * sub : 256 * (sub + 1)],
                    start=(k == 0),
                    stop=(k == NK - 1),
                )

        # Tail: out = exp(0.2 * ln(clip(s, EPS, 1-EPS)))
        s_sb = tailp.tile([B, CP], dt.float32, tag="tail")
        nc.vector.tensor_scalar(
            out=s_sb[:],
            in0=s_ps[:],
            scalar1=EPS,
            scalar2=1.0 - EPS,
            op0=ALU.max,
            op1=ALU.min,
        )
        w = tailp.tile([B, CP], dt.float32, tag="tail")
        nc.scalar.activation(out=w[:], in_=s_sb[:], func=AF.Ln)
        ob = tailp.tile([B, CP], dt.float32, tag="tail")
        nc.scalar.activation(out=ob[:], in_=w[:], func=AF.Exp, scale=1.0 / 5.0)
        nc.sync.dma_start(out=o_d[:], in_=ob[:])

    nc.finalize()
    return nc


def kernel(inputs: np.ndarray, R: np.ndarray) -> np.ndarray:
    from concourse.bass_utils import run_bass_kernel_spmd

    if "nc" not in _STATE:
        _STATE["nc"] = _build_nc()
    nc = _STATE["nc"]

    x = np.ascontiguousarray(inputs, dtype=np.float32)
    in_maps = [
        {"x": x, "r": np.ascontiguousarray(R[i * CP : (i + 1) * CP])}
        for i in range(NCORES)
    ]
    res = run_bass_kernel_spmd(nc, in_maps, core_ids=list(range(NCORES)))
    _STATE["last_results"] = res
    out = np.concatenate([res.results[i]["out"] for i in range(NCORES)], axis=1)
    return np.ascontiguousarray(out, dtype=np.float32)


# revision 4
# speedup vs baseline: 1.3444x; 1.3444x over previous
"""Trainium2 Bass kernel for hierarchical-classification AWX head.

Computes, for inputs x[B, L] (f32) and 0/1 adjacency R[C, L] (int32):

    o   = sigmoid(x)
    s   = einsum('bl,cl->bc', o**5, R)          (R**5 == R since R is 0/1)
    out = clip(s, EPS, 1-EPS) ** (1/5)

Sharding: R is split row-wise (class dim) across the 8 NeuronCores; each
core computes a [B, C/8] slice of the output against the full (replicated)
x. No cross-device reduction is needed; the host concatenates the slices.

Per-core pipeline (all compute on device):
  - x is DMA'd first (it gates the serial ACT front), folded as
    [128, 2048] ((l-half, b) on partitions), in 4 quarter-DMAs so the
    activation chain can start on the first column half early.
  - sigmoid(x)^5 = exp(-5 * ln(1 + exp(-x))): ScalarE passes using only
    Exp/Ln, which share one ACT table set; the table-set choice is pinned
    to natural_log_exp_and_others via a build-time patch so exactly one
    ACT_TABLE_LOAD is paid (warmed up front by a dummy activation).
  - R slice is cast int32->bf16 during the SWDGE DMA load (values 0/1),
    8 chunks all in flight.
  - The matmul contracts over l, so both operands need l on partitions:
    both are transposed on the TensorE via matmul-with-identity
    (out = tile^T @ I), grouped 4 tiles per PSUM bank, evacuated by
    VectorE/ScalarE copies (f32->bf16). A few dummy identity matmuls are
    issued up front to warm the PE HAM clock gate.
  - 32 accumulating bf16 matmuls build s[64, 256] in PSUM.
  - Tail: clip (VectorE two-op tensor_scalar), ln, exp(0.2*) (ScalarE).
"""

import numpy as np

B, L, C = 64, 4096, 2048
NCORES = 8
CP = C // NCORES  # 256 classes per core
EPS = 1e-6

NK = L // 128  # 32 contraction chunks of 128
H = 2          # fold factor for x: [64, 4096] -> [128, 2048]
QW = 1024      # R dma chunk width along l
NQ = L // QW   # 4
COLH = L // H // 2  # 1024: column half of the folded x layout
N_WARMUP_MM = 12

ACT_SET = "natural_log_exp_and_others"

_STATE = {}


def _patch_act_tables():
    """Pin bacc's ACT table-set selection to the one set containing both
    Exp and Ln (plus Copy), so the kernel pays a single ACT_TABLE_LOAD
    instead of thrashing between exp_and_others / natural_log.
    Entry order and count are preserved so act_func_set_id stays aligned
    with the compiler's act_info.json."""
    import functools

    import concourse.bacc as bacc_mod
    import concourse.hw_specs as hw_specs

    if getattr(bacc_mod.get_activation_tables, "_awx_patched", False):
        return

    orig = hw_specs.get_activation_tables

    @functools.cache
    def patched(module_arch):
        tabs = orig(module_arch)
        assert ACT_SET in tabs, sorted(tabs)
        return {
            name: (fns if name == ACT_SET else type(fns)())
            for name, fns in tabs.items()
        }

    patched._awx_patched = True
    bacc_mod.get_activation_tables = patched


def _build_nc():
    from contextlib import ExitStack

    import ml_dtypes
    import concourse.bacc as bacc
    import concourse.mybir as mybir
    from concourse.tile import TileContext

    _patch_act_tables()

    dt = mybir.dt
    AF = mybir.ActivationFunctionType
    ALU = mybir.AluOpType

    nc = bacc.Bacc("TRN2", target_bir_lowering=False)

    x_d = nc.dram_tensor("x", [B, L], dt.float32, kind="ExternalInput")
    r_d = nc.dram_tensor("r", [CP, L], dt.int32, kind="ExternalInput")
    o_d = nc.dram_tensor("out", [B, CP], dt.float32, kind="ExternalOutput")
    ident_d = nc.inline_tensor(np.eye(128, dtype=ml_dtypes.bfloat16), "ident")

    with TileContext(nc) as tc, ExitStack() as ctx:
        const = ctx.enter_context(tc.tile_pool(name="const", bufs=1))
        xin = ctx.enter_context(tc.tile_pool(name="xin", bufs=1))
        actp = ctx.enter_context(tc.tile_pool(name="actp", bufs=4))
        o5p = ctx.enter_context(tc.tile_pool(name="o5p", bufs=2))
        otp = ctx.enter_context(tc.tile_pool(name="otp", bufs=4))
        rbp = ctx.enter_context(tc.tile_pool(name="rbp", bufs=2 * NQ))
        rtp = ctx.enter_context(tc.tile_pool(name="rtp", bufs=4))
        tailp = ctx.enter_context(tc.tile_pool(name="tailp", bufs=3))
        pst = ctx.enter_context(tc.tile_pool(name="pst", bufs=3, space="PSUM"))
        psw = ctx.enter_context(tc.tile_pool(name="psw", bufs=1, space="PSUM"))
        pss = ctx.enter_context(tc.tile_pool(name="pss", bufs=1, space="PSUM"))

        ident = const.tile([128, 128], dt.bfloat16)
        nc.sync.dma_start(out=ident[:], in_=ident_d[:])

        # ACT table warmup: trigger the single ACT_TABLE_LOAD before x
        # arrives, on a tiny memset tile.
        warm_in = const.tile([128, 8], dt.float32)
        nc.gpsimd.memset(warm_in[:], 0.0)
        warm_out = const.tile([128, 8], dt.float32)
        nc.scalar.activation(out=warm_out[:], in_=warm_in[:], func=AF.Exp)

        # x folded: partition p = 64*h + b, free q = l % 2048 (l = 2048h + q).
        # 4 quarter-DMAs; column half ch needs quarters (h=0,ch),(h=1,ch).
        xf = xin.tile([128, L // H], dt.float32)
        for ch in range(2):
            for h in range(H):
                nc.sync.dma_start(
                    out=xf[64 * h : 64 * (h + 1), COLH * ch : COLH * (ch + 1)],
                    in_=x_d[:, (L // H) * h + COLH * ch : (L // H) * h + COLH * (ch + 1)],
                )

        # R slice loads, cast int32 -> bf16 during DMA (SWDGE), all in flight.
        # rb[(t, q)][c', l'] = R[128t + c', QW*q + l'] for this core's slice.
        rb = {}
        for q in range(NQ):
            for t in range(2):
                tile_ = rbp.tile([128, QW], dt.bfloat16, tag="rb")
                nc.gpsimd.dma_start(
                    out=tile_[:],
                    in_=r_d[128 * t : 128 * (t + 1), QW * q : QW * (q + 1)],
                )
                rb[(t, q)] = tile_

        # PE HAM warmup: dummy identity matmuls (dep: ident DMA only) so the
        # clock gate reaches 8/8 before the real transposes arrive.
        ps_w = psw.tile([128, 128], dt.float32)
        for _ in range(N_WARMUP_MM):
            nc.tensor.matmul(
                out=ps_w[:], lhsT=ident[:], rhs=ident[:], start=True, stop=True
            )

        # o5 = sigmoid(x)^5 = exp(-5 * ln(1 + exp(-x))), split into two
        # column halves so the first transposes can start early.
        o5h = []
        for ch in range(2):
            sl = slice(COLH * ch, COLH * (ch + 1))
            t1 = actp.tile([128, COLH], dt.float32, tag="acttmp")
            nc.scalar.activation(out=t1[:], in_=xf[:, sl], func=AF.Exp, scale=-1.0)
            u = actp.tile([128, COLH], dt.float32, tag="acttmp")
            nc.scalar.activation(out=u[:], in_=t1[:], func=AF.Ln, bias=1.0)
            ob5 = o5p.tile([128, COLH], dt.bfloat16, tag="o5")
            nc.scalar.activation(out=ob5[:], in_=u[:], func=AF.Exp, scale=-5.0)
            o5h.append(ob5)

        # --- helpers emitting PE/copy work --------------------------------
        ot = [None] * 4
        copy_count = [0]

        def copyback(dst_ap, src_ap, to_dve):
            if to_dve:
                nc.vector.tensor_copy(out=dst_ap, in_=src_ap)
            else:
                nc.scalar.copy(out=dst_ap, in_=src_ap)

        def emit_o5t(jg):
            # transpose 4 folded o5 tiles (j = 4jg..4jg+3) into one bank
            ps = pst.tile([128, 512], dt.float32, tag="pst")
            for jj in range(4):
                j = 4 * jg + jj
                ch, jc = divmod(j, 8)  # column half, tile within half
                nc.tensor.matmul(
                    out=ps[:, 128 * jj : 128 * (jj + 1)],
                    lhsT=o5h[ch][:, 128 * jc : 128 * (jc + 1)],
                    rhs=ident[:],
                    start=True,
                    stop=True,
                )
            sb = otp.tile([128, 512], dt.bfloat16, tag="ot")
            copyback(sb[:], ps[:], to_dve=(jg % 2 == 0))
            ot[jg] = sb

        rt_tiles = [None] * (NK // 2)

        def emit_rt(kk):
            # transpose R chunks for k = 2kk, 2kk+1 (both c-halves) into one
            # bank; grouped copy to SBUF as the rhs pair.
            ps = pst.tile([128, 512], dt.float32, tag="pst")
            for sub in range(2):
                k = 2 * kk + sub
                q, off = divmod(k * 128, QW)
                for t in range(2):
                    nc.tensor.matmul(
                        out=ps[:, 256 * sub + 128 * t : 256 * sub + 128 * (t + 1)],
                        lhsT=rb[(t, q)][:, off : off + 128],
                        rhs=ident[:],
                        start=True,
                        stop=True,
                    )
            rt = rtp.tile([128, 512], dt.bfloat16, tag="rt")
            copyback(rt[:], ps[:], to_dve=(kk % 4 != 3))
            rt_tiles[kk] = rt

        s_ps = pss.tile([B, CP], dt.float32)

        def emit_main(kk):
            for sub in range(2):
                k = 2 * kk + sub
                j, h = k % 16, k // 16
                jg, jj = divmod(j, 4)
                nc.tensor.matmul(
                    out=s_ps[:],
                    lhsT=ot[jg][:, 128 * jj + 64 * h : 128 * jj + 64 * h + 64],
                    rhs=rt_tiles[kk][:, 256 * sub : 256 * (sub + 1)],
                    start=(k == 0),
                    stop=(k == NK - 1),
                )

        # --- PE schedule ---------------------------------------------------
        emit_rt(0)
        emit_o5t(0)
        emit_o5t(1)
        emit_main(0)
        emit_rt(1)
        emit_main(1)
        emit_rt(2)
        emit_o5t(2)
        emit_o5t(3)
        emit_main(2)
        for kk in range(3, NK // 2):
            emit_rt(kk)
            emit_main(kk)

        # Tail: out = exp(0.2 * ln(clip(s, EPS, 1-EPS)))
        s_sb = tailp.tile([B, CP], dt.float32, tag="tail")
        nc.vector.tensor_scalar(
            out=s_sb[:],
            in0=s_ps[:],
            scalar1=EPS,
            scalar2=1.0 - EPS,
            op0=ALU.max,
            op1=ALU.min,
        )
        w = tailp.tile([B, CP], dt.float32, tag="tail")
        nc.scalar.activation(out=w[:], in_=s_sb[:], func=AF.Ln)
        ob = tailp.tile([B, CP], dt.float32, tag="tail")
        nc.scalar.activation(out=ob[:], in_=w[:], func=AF.Exp, scale=1.0 / 5.0)
        nc.sync.dma_start(out=o_d[:], in_=ob[:])

    nc.finalize()
    return nc


def kernel(inputs: np.ndarray, R: np.ndarray) -> np.ndarray:
    from concourse.bass_utils import run_bass_kernel_spmd

    if "nc" not in _STATE:
        _STATE["nc"] = _build_nc()
    nc = _STATE["nc"]

    x = np.ascontiguousarray(inputs, dtype=np.float32)
    in_maps = [
        {"x": x, "r": np.ascontiguousarray(R[i * CP : (i + 1) * CP])}
        for i in range(NCORES)
    ]
    res = run_bass_kernel_spmd(nc, in_maps, core_ids=list(range(NCORES)))
    _STATE["last_results"] = res
    out = np.concatenate([res.results[i]["out"] for i in range(NCORES)], axis=1)
    return np.ascontiguousarray(out, dtype=np.float32)
